# revision 51
# baseline (speedup 1.0000x reference)
"""Causal single-head attention (B=4, T=4096, C=1024, D=64) on 8 NeuronCores.

Sharding: core c = (batch b = c % 4, half h = c // 4).
Each core handles ALL queries of its batch, but only its half of the key
blocks (256-token key blocks with block index ≡ h mod 2).  Pure SPMD; cores
differ only in input data.  Each core emits unnormalized partial results
U^T = [V|1]^T @ exp(S^T) per query supertile; the host combines the two
halves per batch: O = (U0 + U1)[:64] / (U0 + U1)[64].

v3 I/O + schedule (vs v1):
  * Single streamed input: x^T pre-tiled on host into 8 supertile blocks
    [C, 512], each column-rotated by 256*h so THIS core's key columns are
    always the first 256 of every 512-column chunk.  K/V projections slice
    keys straight out of the query stream - no separate xk input.  Masks
    and outputs are correspondingly permuted host-side.
  * Projections col-packed: two 64-row outputs run concurrently in the two
    PE col groups (Q of two supertiles; K and V of one block).
  * Scores row-packed: qT/kT duplicated on both partition halves so two
    128-key tiles run concurrently in PE row groups (K=64 each).
  * Everything software-pipelined in emission order; the ScalarE exp chain
    (36 x ~1.1us) paces the kernel, PE/DMA/DVE hide underneath.
"""
import sys
from collections import deque
import numpy as np
import ml_dtypes

if "/opt/trn_rl_repo" not in sys.path:
    sys.path.insert(0, "/opt/trn_rl_repo")

import concourse.bacc as bacc
import concourse.mybir as mybir
from concourse import tile
from concourse import bass_utils

bf16 = mybir.dt.bfloat16
f32 = mybir.dt.float32
BF = ml_dtypes.bfloat16

B, T, C, D = 4, 4096, 1024, 64
NST = 8          # query supertiles per batch (512 queries each)
STQ = 512
TK = T // 2      # key tokens per core
NKT = TK // 128  # local 128-key tiles per core (16)
NKB = TK // 512  # local 512-key blocks per core (4)
NC_ = C // 128   # 8 c-tiles

_CACHE = {}


def _build():
    nc = bacc.Bacc(None, target_bir_lowering=False, debug=False, num_devices=8)

    # x^T pre-tiled to SBUF layout in four DRAM-contiguous pieces per
    # supertile block: row 512*st + 128*pc + p holds partition p of piece
    # pc (c-chunks 2pc, 2pc+1).  Columns of each supertile are per-core
    # rotated so cols [0:256) of every 512-column chunk are THIS core's
    # key tokens.
    xq = nc.dram_tensor("xq", [NST * 4 * 128, 1024], bf16,
                        kind="ExternalInput")
    w = nc.dram_tensor("w", [128, NC_ * 192], bf16,
                       kind="ExternalInput")   # Wq|Wk|Wv, c-chunks packed
    msk = nc.dram_tensor("msk", [256, STQ], bf16, kind="ExternalInput")
    idn = nc.dram_tensor("idn", [65, 65], bf16, kind="ExternalInput")
    idb = nc.dram_tensor("idb", [128, 128], bf16, kind="ExternalInput")
    out = nc.dram_tensor("out", [65, T], f32, kind="ExternalOutput")

    with tile.TileContext(nc) as tc:
        with tc.tile_pool(name="sb", bufs=1) as sb, \
             tc.tile_pool(name="xqp", bufs=5) as xqp, \
             tc.tile_pool(name="pp", bufs=3) as pp, \
             tc.tile_pool(name="usp", bufs=2) as usp, \
             tc.tile_pool(name="ps_s", bufs=2, space="PSUM") as ps_s, \
             tc.tile_pool(name="ps_u", bufs=2, space="PSUM") as ps_u, \
             tc.tile_pool(name="ps_a", bufs=2, space="PSUM") as ps_a:

            # ---- small resident inputs ----
            # small resident inputs ride the (otherwise idle-at-start)
            # ScalarE DMA queue; SyncE is reserved for the first two x
            # blocks and outputs; GpSimdE streams the remaining x blocks.
            w_t = sb.tile([128, NC_ * 192], bf16, tag="w")
            msk_t = sb.tile([128, 2 * STQ], bf16, tag="msk")
            idn_t = sb.tile([65, 65], bf16, tag="idn")
            idb_t = sb.tile([128, 128], bf16, tag="idb")
            nc.scalar.dma_start(w_t[:], w[:])
            nc.scalar.dma_start(msk_t[:, 0:STQ], msk[0:128, :])
            nc.scalar.dma_start(msk_t[:, STQ:2 * STQ], msk[128:256, :])
            nc.scalar.dma_start(idn_t[:], idn[:])
            nc.scalar.dma_start(idb_t[:], idb[:])

            # ---- persistent intermediates ----
            # qT2/kT2: transposed projections duplicated on both partition
            # halves so row-packed score matmuls can source row group 64-127.
            qT2 = sb.tile([128, T], bf16, tag="qT2")
            kT2 = sb.tile([128, TK], bf16, tag="kT2")
            vT = sb.tile([65, TK], bf16, tag="vT")   # row 64 = ones
            vP = sb.tile([128, NKT * 65], bf16, tag="vP")  # V tiles [key, d|1]
            nc.vector.memset(vT[64:65, :], 1.0)

            # ---- streamed input blocks: [128, (c, col)] layout ----
            xqb = {}

            def dma_xq(st, eng=None):
                # four 256KB fully-contiguous descriptors per block; later
                # blocks issue from the idle GpSimd engine so the SyncE
                # dma_start issue rate (~0.6us each) isn't a serial
                # bottleneck for the early stream
                t_ = xqp.tile([128, NC_ * STQ], bf16, tag="xqb",
                              name=f"xqb{st}")
                xqb[st] = t_
                eng = eng or nc.gpsimd
                for pc in range(4):
                    eng.dma_start(
                        t_[:, 1024 * pc:1024 * (pc + 1)],
                        xq[512 * st + 128 * pc:512 * st + 128 * (pc + 1), :])

            # ---- projection emitters ----
            # Col-packed: the two PE col groups compute two independent
            # 64-row outputs concurrently (out partitions 0-63 / 64-127),
            # each accumulating over all 8 C-chunks.
            def q_proj2_items(stp):
                """Q^T for supertiles 2*stp (col group 0) and 2*stp+1
                (col group 1): same Wq weights, different moving operand."""
                st0, st1 = 2 * stp, 2 * stp + 1
                acc = ps_a.tile([128, STQ], f32, tag="acc",
                                name=f"qacc{stp}")
                items = []
                for c in range(NC_):
                    def mm(c=c, acc=acc):
                        for hh, st in ((0, st0), (1, st1)):
                            nc.tensor.matmul(
                                acc[64 * hh:64 * (hh + 1), :],
                                w_t[:, 192 * c:192 * c + 64],
                                xqb[st][:, 512 * c:512 * (c + 1)],
                                start=(c == 0), stop=(c == NC_ - 1))
                    items.append(mm)

                def fin(hh, st, acc=acc):
                    qsl = slice(STQ * st, STQ * (st + 1))
                    src = acc[64 * hh:64 * (hh + 1), :]
                    nc.vector.tensor_copy(qT2[0:64, qsl], src)
                    nc.vector.tensor_copy(qT2[64:128, qsl], src)
                items.append(lambda: fin(0, st0))
                items.append(lambda: fin(1, st1))
                return items

            def kv_proj2_items(b):
                """K^T and V^T for local key block b = the leading
                256-column key halves of supertile blocks 2b and 2b+1.
                Col-packed per tensor: piece 0 (keys 512b+[0,256)) in col
                group 0, piece 1 in col group 1 (partition-split groups
                have well-defined per-half PSUM accumulation)."""
                def pidx(st, k):
                    return st * (st + 1) // 2 + k

                # first read: diag pair (minus one pair of DVE-latency slack)
                dl = max(pidx(2 * b, 2 * b) - 1, 1)
                accK = ps_a.tile([128, 256], f32, tag="acc", name=f"kacc{b}")
                accV = ps_a.tile([128, 256], f32, tag="acc", name=f"vacc{b}")
                items = []
                for c in range(NC_):
                    for kk in range(2):
                        # complementary col groups so the K and V matmuls
                        # run concurrently: (K piece kk, V piece 1-kk)
                        def mm(c=c, kk=kk, accK=accK, accV=accV):
                            for acc, wofs, hh in ((accK, 64, kk),
                                                  (accV, 128, 1 - kk)):
                                nc.tensor.matmul(
                                    acc[64 * hh:64 * (hh + 1), :],
                                    w_t[:, 192 * c + wofs:192 * c + wofs + 64],
                                    xqb[2 * b + hh][:, 512 * c:512 * c + 256],
                                    start=(c == 0), stop=(c == NC_ - 1))
                        items.append((dl, mm))

                def finV(hh, accV=accV):
                    ksl = slice(512 * b + 256 * hh, 512 * b + 256 * (hh + 1))
                    nc.vector.tensor_copy(vT[0:64, ksl],
                                          accV[64 * hh:64 * (hh + 1), :])

                def finK(accK=accK):
                    for hh in range(2):
                        ksl = slice(512 * b + 256 * hh,
                                    512 * b + 256 * (hh + 1))
                        src = accK[64 * hh:64 * (hh + 1), :]
                        nc.vector.tensor_copy(kT2[0:64, ksl], src)
                        nc.vector.tensor_copy(kT2[64:128, ksl], src)

                def tr(j):
                    tp = ps_a.tile([128, 65], bf16, tag="acc", name=f"tp{j}")
                    nc.tensor.transpose(tp[:], vT[:, 128 * j:128 * (j + 1)],
                                        idn_t[:])
                    nc.vector.tensor_copy(vP[:, 65 * j:65 * (j + 1)], tp[:])

                # V copies feed the transposes; interleave so PE only waits
                # for one [64,256] DVE copy, and K copies ride along last.
                dl1 = pidx(2 * b + 1, 2 * b + 1) - 1
                items.append((dl, lambda: finV(0)))
                items.append((dl, lambda: tr(4 * b)))
                items.append((dl, lambda: (finV(1), finK())))
                items.append((dl, lambda: tr(4 * b + 1)))
                items.append((dl1, lambda: tr(4 * b + 2)))
                items.append((dl1, lambda: tr(4 * b + 3)))
                return items

            # ---- attention pair stream ----
            pairs = [(st, k) for st in range(NST) for k in range(st + 1)]
            s2_of = {}
            p2_of = {}
            u_of = {}

            def scores(i):
                st, k = pairs[i]
                qsl = slice(STQ * st, STQ * (st + 1))
                s2 = ps_s.tile([128, 2 * STQ], f32, tag="s", name=f"s{i}")
                s2_of[i] = s2
                diag = (k == st)
                j0, j1 = 2 * k, 2 * k + 1
                nc.tensor.matmul(s2[:, 0:STQ],
                                 kT2[0:64, 128 * j0:128 * (j0 + 1)],
                                 qT2[0:64, qsl], start=True, stop=not diag)
                nc.tensor.matmul(s2[:, STQ:2 * STQ],
                                 kT2[64:128, 128 * j1:128 * (j1 + 1)],
                                 qT2[64:128, qsl], start=True, stop=not diag)
                if diag:
                    # causal mask as a -1e9 additive bias (identity-weight
                    # matmul accumulating into PSUM) - keeps DVE off the
                    # scores -> exp -> PV critical chain
                    nc.tensor.matmul(s2[:, 0:STQ], idb_t[:],
                                     msk_t[:, 0:STQ],
                                     start=False, stop=True)
                    nc.tensor.matmul(s2[:, STQ:2 * STQ], idb_t[:],
                                     msk_t[:, STQ:2 * STQ],
                                     start=False, stop=True)

            def exp_mask(i):
                p2 = pp.tile([128, 2 * STQ], bf16, tag="p", name=f"p{i}")
                p2_of[i] = p2
                nc.scalar.activation(p2[:], s2_of[i][:],
                                     mybir.ActivationFunctionType.Exp,
                                     scale=0.125)
                del s2_of[i]

            def pv(i):
                st, k = pairs[i]
                if k == 0:
                    u_of[st] = ps_u.tile([65, STQ], f32, tag="u",
                                         name=f"u{st}")
                u = u_of[st]
                p2 = p2_of.pop(i)
                for dd in range(2):
                    j = 2 * k + dd
                    nc.tensor.matmul(u[:], vP[:, 65 * j:65 * (j + 1)],
                                     p2[:, STQ * dd:STQ * (dd + 1)],
                                     start=(j == 0), stop=(j == 2 * st + 1))

            def drain_u(st):
                qsl = slice(STQ * st, STQ * (st + 1))
                u_sb = usp.tile([65, STQ], f32, tag="usb", name=f"usb{st}")
                nc.vector.tensor_copy(u_sb[:], u_of.pop(st)[:])
                nc.sync.dma_start(out[:, qsl], u_sb[:])

            # ---- emission schedule ----
            def P(st, k):   # global pair index
                return st * (st + 1) // 2 + k

            bg = deque()

            # HAM warm-up: a dense stream of cheap matmuls from ~t=2us keeps
            # the PE clock-gate busy through the DMA ramp so the real
            # preamble runs at 2.4 GHz instead of 1.2.
            scr_w = sb.tile([128, 32], bf16, tag="scrw")
            scr_r = sb.tile([128, 128], bf16, tag="scrr")
            nc.vector.memset(scr_w[:], 0.0)
            nc.vector.memset(scr_r[:], 0.0)
            # trigger the exp table-set DMA (~2.7us) at t=0, off the
            # critical path
            nc.scalar.activation(scr_r[:, 0:32], scr_w[:],
                                 mybir.ActivationFunctionType.Exp)
            dmy = ps_a.tile([32, 128], f32, tag="acc", name="dmy")

            def dummies(n):
                # cheap PE no-ops: keep the HAM activity monitor busy while
                # real matmuls wait on streaming DMA, so the clock stays at
                # 2.4 GHz instead of sagging to 1.2
                for _ in range(n):
                    nc.tensor.matmul(dmy[:, 0:96], scr_w[:],
                                     scr_r[:, 32:128],
                                     start=True, stop=True)

            dma_xq(0, eng=nc.sync)
            dma_xq(1, eng=nc.sync)
            for st_ in range(2, NST):
                dma_xq(st_)   # gpsimd queue, self-paced by pool slots
            # one warm-up block sized to cover the xqb0/1 DMA window
            dummies(88)
            q0 = q_proj2_items(0)
            kv0 = [it for _, it in kv_proj2_items(0)]
            for c in range(NC_):
                q0[c]()
                kv0[2 * c]()
                kv0[2 * c + 1]()
            for it in q0[NC_:]:
                it()
            for it in kv0[2 * NC_:]:
                it()

            def q_bg(stp):
                # 8 mm chunks + fin(st=2stp) due before its first pair;
                # fin(st=2stp+1) a supertile later
                its = q_proj2_items(stp)
                d0 = max(P(2 * stp, 0) - 1, 1)
                d1 = max(P(2 * stp + 1, 0) - 1, 1)
                return [(d0, it) for it in its[:-1]] + [(d1, its[-1])]

            # st -> generator of (deadline_pair_idx, item): items must be
            # fully EMITTED before the score of that pair is emitted (Tile
            # deps follow trace order - a consumer traced before its
            # producer reads garbage).  Q feeds the first pair of its
            # supertile; K/V tiles are first read by their diagonal pair.
            work_plan = {
                0: lambda: q_bg(1),
                1: lambda: kv_proj2_items(1),
                2: lambda: q_bg(2),
                3: lambda: kv_proj2_items(2),
                4: lambda: q_bg(3),
                5: lambda: kv_proj2_items(3),
            }

            scores(0)
            for i, (st, k) in enumerate(pairs):
                if k == 0 and st in work_plan:
                    bg.extend(work_plan[st]())
                if i + 1 < len(pairs):
                    while bg and bg[0][0] <= i + 1:   # due before next pair
                        bg.popleft()[1]()
                    scores(i + 1)
                exp_mask(i)
                pv(i)
                if k == st:
                    drain_u(st)
                # keep PE fed just below the ~1.1us ScalarE exp per pair:
                # fine-grained items (~0.1-0.2us each)
                budget = 3 if (bg and bg[0][0] <= i + 2) else 2
                for _ in range(budget):
                    if bg:
                        bg.popleft()[1]()
            while bg:
                bg.popleft()[1]()

    nc.compile()
    return nc


def _get_nc():
    if "nc" not in _CACHE:
        _CACHE["nc"] = _build()
    return _CACHE["nc"]


def kernel(x, Wq, Wk, Wv, _trace=False):
    x = np.asarray(x)
    nc = _get_nc()

    xT = np.ascontiguousarray(x.transpose(0, 2, 1)).astype(BF)   # [B, C, T]
    w = np.concatenate([Wq, Wk, Wv], axis=1).astype(BF)          # [C, 192]
    # pack to device layout [128, (c, k)]: row p, col 192*c+k = w[128c+p, k]
    w2 = np.ascontiguousarray(
        w.reshape(NC_, 128, 192).transpose(1, 0, 2)).reshape(128, NC_ * 192)
    idn = np.eye(65, dtype=BF)

    # Column-rotated supertile blocks in device DMA layout: four contiguous
    # pieces per supertile, piece pc = partitions x c-chunks (2pc, 2pc+1).
    # Core (b, h) sees supertile st with columns rolled left by 256*h, so
    # its key half is always cols [0:256) of each 512-column chunk.
    xqs = {}
    for bidx in range(B):
        blocks = xT[bidx].reshape(C, NST, STQ).transpose(1, 0, 2)  # [st,C,q]
        for h in range(2):
            rb = np.roll(blocks, -256 * h, axis=2) if h else blocks
            # [st, C, q] -> [st, pc, chalf, p, q] -> [st, pc, p, chalf, q]
            xqs[(bidx, h)] = np.ascontiguousarray(
                rb.reshape(NST, 4, 2, 128, STQ).transpose(0, 1, 3, 2, 4)
            ).reshape(NST * 4 * 128, 1024)

    # Causal masks as additive bias (0 keep / -1e9 drop) in permuted query
    # coordinates: query column j of a supertile is global offset
    # (j + 256h) % 512; diag tile d covers keys 256h+128d+r.
    jj = np.arange(STQ)[None, :]
    rr = np.arange(128)[:, None]
    masks = {}
    for h in range(2):
        gq = (jj + 256 * h) % 512
        m0 = np.where(rr <= gq - 256 * h, 0.0, -1e9).astype(BF)
        m1 = np.where(rr <= gq - 256 * h - 128, 0.0, -1e9).astype(BF)
        masks[h] = np.concatenate([m0, m1], axis=0)
    idb = np.eye(128, dtype=BF)

    in_maps = []
    for cid in range(8):
        bidx, h = cid % 4, cid // 4
        in_maps.append({
            "xq": xqs[(bidx, h)],
            "w": w2,
            "msk": masks[h],
            "idn": idn,
            "idb": idb,
        })

    res = bass_utils.run_bass_kernel_spmd(nc, in_maps, core_ids=list(range(8)),
                                          trace=_trace)
    _CACHE["last_results"] = res

    O = np.empty((B, T, D), dtype=np.float32)
    for bidx in range(B):
        U = np.zeros((65, T), dtype=np.float32)
        for h in range(2):
            part = res.results[bidx + 4 * h]["out"]        # [65, T] permuted
            blocks = part.reshape(65, NST, STQ)
            U += np.roll(blocks, 256 * h, axis=2).reshape(65, T)
        O[bidx] = (U[:D] / U[D:D + 1]).T
    return O
